# revision 18
# baseline (speedup 1.0000x reference)
"""Trainium2 Bass kernel for DY_Block (EfficientAT DyMN dynamic block).

Data-parallel over batch B=32 across 8 NeuronCores (4 samples/core); all
intermediates stay on-chip between input load and output store.

Per core:
  Phase A (batched over the 4 local samples):
    means -> ContextGen joint conv (PE; BN + mean-div folded into lhsT) ->
    hswish -> g_c -> routing softmax (PE/ACT/DVE) -> att transposed (PE) and
    partition-broadcast (GPSIMD) -> DyReLU coefs (PE matmul + ACT sigmoid,
    transposed to channel-partition tiles) -> CoordAtt gates (PE + ACT), bf16.
  Phase B per sample:
    mix expert weights (DVE STT chains) -> expansion conv (PE bf16) ->
    BN+hswish (ACT relu + DVE) into padded [c,(f,t)] layout -> depthwise 3x3
    as 9-tap DVE STT chain (shifts = AP offsets) -> DyReLU (ACT FMA x2 + DVE
    max) -> CoordAtt gating (DVE, broadcast views) -> projection conv (PE,
    accumulating) -> BN bias (ACT) + residual (DVE) -> DMA out.
"""
import ml_dtypes
import numpy as np

import concourse.bass as bass
import concourse.bacc as bacc
import concourse.tile as tile
from concourse import mybir
from concourse.bass_utils import run_bass_kernel_spmd

F32 = mybir.dt.float32
BF16 = mybir.dt.bfloat16
AX = mybir.AxisListType
OP = mybir.AluOpType
AF = mybir.ActivationFunctionType

B, CIN, CEXP, COUT, F, T = 32, 80, 480, 80, 32, 125
CTX, K, M = 120, 4, 2
TEMP = 30.0
EPS = 1e-3
NCORES = 8
S = B // NCORES          # samples per core
NBLK = CEXP // CTX       # 4 channel blocks of 120
FT = F * T               # 4000
TP = 128                 # padded row stride for depthwise layout
FP = F + 2               # padded f rows
XOFF = 4                 # even lead offset so tap views never start below 0
NPAD = XOFF + FP * TP + 4  # padded per-block free size (+ slack for +1,+1 tap)
TOFF = 2                 # t offset inside padded row (even, for bf16 align)
NCH = 8                  # N-chunks per (blk) matmul pass
CHW = FT // NCH          # 500 cols per chunk
CHF = F // NCH           # 4 f rows per chunk


def _emit(tc, io, ctx):
    nc = tc.nc

    (xs, jlf, jlt, jb3, cvf, cvt, bfb, btb, arw, drw, expw, eb3,
     depw, bnbd, projw, pbias, identf, identb, ones1, yout) = io

    wpool = ctx.enter_context(tc.tile_pool(name="weights", bufs=1))
    ctx_pool = ctx.enter_context(tc.tile_pool(name="ctx", bufs=1))
    ps_mm = ctx.enter_context(tc.tile_pool(name="ps_mm", bufs=2, space="PSUM"))
    ps_small = ps_mm
    work = ctx.enter_context(tc.tile_pool(name="work", bufs=2))
    zpool = ctx.enter_context(tc.tile_pool(name="zpool", bufs=1))
    xpool = ctx.enter_context(tc.tile_pool(name="xpool", bufs=2))

    # ---------- persistent weights ----------
    def wtile(ap, tag):
        t = wpool.tile(list(ap.shape), ap.dtype, tag=tag)
        nc.sync.dma_start(t[:], ap)
        return t

    w_jlf = wtile(jlf, "jlf")          # [80,120]
    w_jlt = wtile(jlt, "jlt")          # [80,120]
    w_jb3 = wtile(jb3, "jb3")          # [120,1]
    w_cvf = wtile(cvf, "cvf")          # [120,480]
    w_cvt = wtile(cvt, "cvt")          # [120,480]
    w_bfb = wtile(bfb, "bfb")          # [120,4]
    w_btb = wtile(btb, "btb")          # [120,4]
    w_arw = wtile(arw, "arw")          # [121,12] (last row = bias)
    w_drw = wtile(drw, "drw")          # [121,1920] (last row = bias)
    w_expw = wtile(expw, "expw")       # [80,1920]
    w_eb3 = wtile(eb3, "eb3")          # [120,4]
    w_depw = wtile(depw, "depw")       # [120,144]
    w_bnbd = wtile(bnbd, "bnbd")       # [120,4]
    w_projw = wtile(projw, "projw")    # [120,1280]
    w_pbias = wtile(pbias, "pbias")    # [80,1]
    w_idf = wtile(identf, "idf")       # [128,128] f32 identity
    w_idb = wtile(identb, "idb")       # [128,128] bf16 identity

    # ---------- Phase A : batched context ----------
    ga_in = ctx_pool.tile([CIN, S * (F + T)], F32)
    for s in range(S):
        for ch in range(NCH):
            x0a = xpool.tile([CIN, CHW], F32, tag="x0s")
            nc.sync.dma_start(x0a[:], xs[s][:, ch * CHW:(ch + 1) * CHW])
            nc.vector.tensor_reduce(
                ga_in[:, s * F + ch * CHF:(s * F) + (ch + 1) * CHF],
                x0a.rearrange("p (f t) -> p f t", f=CHF), AX.X, OP.add)
            ctc = ga_in[:, S * F + s * T: S * F + (s + 1) * T]
            if ch == 0:
                nc.vector.tensor_reduce(
                    ctc, x0a.rearrange("p (f t) -> p t f", f=CHF), AX.X, OP.add)
            else:
                ctp = work.tile([CIN, T], F32, tag="ctp")
                nc.vector.tensor_reduce(
                    ctp[:], x0a.rearrange("p (f t) -> p t f", f=CHF), AX.X, OP.add)
                nc.vector.tensor_add(ctc, ctc, ctp[:])

    ps_g1 = ps_small.tile([CTX, S * F], F32, tag="pse")
    nc.tensor.matmul(ps_g1[:], w_jlf[:], ga_in[:, 0:S * F], start=True, stop=True)
    ps_g2 = ps_small.tile([CTX, S * T], F32, tag="pse")
    nc.tensor.matmul(ps_g2[:], w_jlt[:], ga_in[:, S * F:], start=True, stop=True)

    r_g = ctx_pool.tile([CTX, S * (F + T)], F32)
    nc.scalar.activation(r_g[:, 0:S * F], ps_g1[:], AF.Relu, bias=w_jb3[:, 0:1], scale=1.0)
    nc.scalar.activation(r_g[:, S * F:], ps_g2[:], AF.Relu, bias=w_jb3[:, 0:1], scale=1.0)
    v_g = ctx_pool.tile([CTX, S * (F + T)], F32)
    nc.vector.tensor_scalar(v_g[:], r_g[:], 6.0, 1.0 / 6.0, OP.min, OP.mult)
    gc_t = r_g  # in-place: (r-3)*v overwrites r
    nc.vector.scalar_tensor_tensor(gc_t[:], r_g[:], -3.0, v_g[:], OP.add, OP.mult)

    g_c = ctx_pool.tile([CTX + 1, S], F32)
    tmp_r = ctx_pool.tile([CTX, S], F32)
    nc.vector.tensor_reduce(
        g_c[0:CTX, :], gc_t[:, 0:S * F].rearrange("p (s f) -> p s f", s=S),
        AX.X, OP.add)
    nc.vector.tensor_reduce(
        tmp_r[:], gc_t[:, S * F:].rearrange("p (s t) -> p s t", s=S), AX.X, OP.add)
    nc.vector.tensor_add(g_c[0:CTX, :], g_c[0:CTX, :], tmp_r[:])
    nc.sync.dma_start(g_c[CTX:CTX + 1, :], ones1)

    # routing attention
    ps_a = ps_small.tile([S, 3 * K], F32, tag="pse")
    nc.tensor.matmul(ps_a[:], g_c[:], w_arw[:], start=True, stop=True)
    ex_t = ctx_pool.tile([S, 3 * K], F32)
    nc.scalar.activation(ex_t[:], ps_a[:], AF.Exp)
    s3 = ctx_pool.tile([S, 3], F32)
    nc.vector.tensor_reduce(
        s3[:], ex_t.rearrange("p (j k) -> p j k", j=3), AX.X, OP.add)
    rec3 = ctx_pool.tile([S, 3], F32)
    nc.vector.reciprocal(rec3[:], s3[:])
    attn = ctx_pool.tile([S, 3 * K], F32)
    for j in range(3):
        nc.vector.tensor_scalar(
            attn[:, j * K:(j + 1) * K], ex_t[:, j * K:(j + 1) * K],
            rec3[:, j:j + 1], None, OP.mult)
    # att48[s, (jk, s')] = attn[s, jk] * I[s, s']; ones-matmul broadcasts to
    # all partitions: att_b[p, (jk, s)] = attn[s, jk]
    att48 = ctx_pool.tile([S, 3 * K * S], F32)
    nc.vector.tensor_tensor(
        att48.rearrange("p (jk s) -> p jk s", s=S),
        attn.unsqueeze(2).broadcast_to((S, 3 * K, S)),
        w_idf[0:S, 0:S].unsqueeze(1).broadcast_to((S, 3 * K, S)),
        OP.mult)
    onesS = ctx_pool.tile([S, CTX], F32)
    nc.vector.memset(onesS[:], 1.0)
    ps_ab = ps_small.tile([CTX, 3 * K * S], F32, tag="pse")
    nc.tensor.matmul(ps_ab[:], onesS[:], att48[:], start=True, stop=True)
    att_b = ctx_pool.tile([CTX, 3 * K * S], F32)
    nc.scalar.copy(att_b[:], ps_ab[:])

    # DyReLU coefficients
    coefs = ctx_pool.tile([S, 2 * M * CEXP], F32)
    for j in range(4):
        ps_th = ps_small.tile([S, CEXP], F32, tag="pse")
        nc.tensor.matmul(ps_th[:], g_c[:], w_drw[:, j * CEXP:(j + 1) * CEXP],
                         start=True, stop=True)
        nc.scalar.activation(coefs[:, j * CEXP:(j + 1) * CEXP], ps_th[:], AF.Sigmoid)
    # theta' = 2*sig - 1; a1 = theta'+1 = 2*sig; a2 = 2*sig-1; b = sig-0.5
    for j, (sc, of) in enumerate([(2.0, 0.0), (2.0, -1.0), (1.0, -0.5), (1.0, -0.5)]):
        nc.vector.tensor_scalar(coefs[:, j * CEXP:(j + 1) * CEXP],
                                coefs[:, j * CEXP:(j + 1) * CEXP],
                                sc, of, OP.mult, OP.add)
    cj = []
    for j in range(4):
        cj_t = ctx_pool.tile([CTX, NBLK * S], F32, tag=f"cj{j}")
        for blk in range(NBLK):
            ps_c = ps_small.tile([CTX, S], F32, tag="pse")
            nc.tensor.transpose(
                ps_c[:], coefs[:, j * CEXP + blk * CTX: j * CEXP + (blk + 1) * CTX],
                w_idf[0:S, 0:S])
            nc.scalar.copy(cj_t[:, blk * S:(blk + 1) * S], ps_c[:])
        cj.append(cj_t)
    for i in range(2):  # fold dep-BN bias: b'_i = a_i*bnb + b_i
        for blk in range(NBLK):
            sl = slice(blk * S, (blk + 1) * S)
            nc.vector.scalar_tensor_tensor(
                cj[2 + i][:, sl], cj[i][:, sl], w_bnbd[:, blk:blk + 1],
                cj[2 + i][:, sl], OP.mult, OP.add)

    # CoordAtt gates (bf16)
    sigf = ctx_pool.tile([CTX, NBLK * S * F], BF16)
    sigt = ctx_pool.tile([CTX, NBLK * S * T], BF16)
    for blk in range(NBLK):
        ps_f = ps_small.tile([CTX, S * F], F32, tag="pse")
        nc.tensor.matmul(ps_f[:], w_cvf[:, blk * CTX:(blk + 1) * CTX],
                         gc_t[:, 0:S * F], start=True, stop=True)
        nc.scalar.activation(sigf[:, blk * S * F:(blk + 1) * S * F], ps_f[:],
                             AF.Sigmoid, bias=w_bfb[:, blk:blk + 1], scale=1.0)
        ps_t2 = ps_small.tile([CTX, S * T], F32, tag="pse")
        nc.tensor.matmul(ps_t2[:], w_cvt[:, blk * CTX:(blk + 1) * CTX],
                         gc_t[:, S * F:], start=True, stop=True)
        nc.scalar.activation(sigt[:, blk * S * T:(blk + 1) * S * T], ps_t2[:],
                             AF.Sigmoid, bias=w_btb[:, blk:blk + 1], scale=1.0)

    # ---------- Phase B : per-sample heavy pipeline ----------

    for s in range(S):
        x0b = xpool.tile([CIN, FT], BF16, tag="x0b", bufs=1)
        for ch in range(NCH):
            x0a = xpool.tile([CIN, CHW], F32, tag="x0s")
            nc.sync.dma_start(x0a[:], xs[s][:, ch * CHW:(ch + 1) * CHW])
            nc.vector.tensor_copy(x0b[:, ch * CHW:(ch + 1) * CHW], x0a[:])

        def mix(dst, src_sl, jr, parts):
            for k in range(K):
                c0 = (jr * K + k) * S + s
                a_col = att_b[0:parts, c0:c0 + 1]
                if k == 0:
                    nc.vector.tensor_scalar(dst, src_sl(k), a_col, None, OP.mult)
                else:
                    nc.vector.scalar_tensor_tensor(dst, src_sl(k), a_col, dst,
                                                   OP.mult, OP.add)

        we = work.tile([CIN, CEXP], F32, tag="we")
        mix(we[:], lambda k: w_expw[:, k * CEXP:(k + 1) * CEXP], 0, CIN)
        web = work.tile([CIN, CEXP], BF16, tag="web")
        nc.vector.tensor_copy(web[:], we[:])

        wd = work.tile([CTX, NBLK * 9], F32, tag="wd")
        for blk in range(NBLK):
            mix(wd[:, blk * 9:(blk + 1) * 9],
                lambda k: w_depw[:, (blk * K + k) * 9:(blk * K + k + 1) * 9], 1, CTX)

        wp = work.tile([CTX, NBLK * COUT], F32, tag="wp")
        for blk in range(NBLK):
            mix(wp[:, blk * COUT:(blk + 1) * COUT],
                lambda k: w_projw[:, (blk * K + k) * COUT:(blk * K + k + 1) * COUT],
                2, CTX)
        wpb = work.tile([CTX, NBLK * COUT], BF16, tag="wpb")
        nc.vector.tensor_copy(wpb[:], wp[:])

        zs = []
        for blk in range(NBLK):
            r_blk = work.tile([CTX, FT], BF16, tag="sa")
            for ch in range(NCH):
                ps_e = ps_mm.tile([CTX, CHW], F32, tag="pse")
                nc.tensor.matmul(ps_e[:], web[:, blk * CTX:(blk + 1) * CTX],
                                 x0b[:, ch * CHW:(ch + 1) * CHW],
                                 start=True, stop=True)
                nc.scalar.activation(r_blk[:, ch * CHW:(ch + 1) * CHW], ps_e[:],
                                     AF.Relu, bias=w_eb3[:, blk:blk + 1], scale=1.0)
            v_blk = work.tile([CTX, FT], BF16, tag="sb")
            nc.vector.tensor_scalar(v_blk[:], r_blk[:], 6.0, 1.0 / 6.0, OP.min, OP.mult)
            xe = work.tile([CTX, NPAD], BF16, tag="xe")
            # zero only the pad regions (lead row, tail row, t-pad columns)
            nc.vector.memset(xe[:, 0:XOFF + TP], 0.0)
            nc.vector.memset(xe[:, XOFF + (F + 1) * TP:NPAD], 0.0)
            xep = xe[:, XOFF:XOFF + FP * TP]
            nc.vector.memset(
                xep.rearrange("p (f t) -> p f t", t=TP)[:, 1:1 + F, 0:TOFF], 0.0)
            nc.vector.memset(
                xep.rearrange("p (f t) -> p f t", t=TP)[:, 1:1 + F, TOFF + T:TP], 0.0)
            xe3 = xep.rearrange("p (f t) -> p f t", t=TP)
            nc.vector.scalar_tensor_tensor(
                xe3[:, 1:1 + F, TOFF:TOFF + T],
                r_blk.rearrange("p (f t) -> p f t", t=T), -3.0,
                v_blk.rearrange("p (f t) -> p f t", t=T), OP.add, OP.mult)

            # DVE partial: the 3 aligned taps (dt=0)
            dacc = work.tile([CTX, F * TP], BF16, tag="dacc")
            base = XOFF + TP
            for i, df in enumerate((-1, 0, 1)):
                ti = (df + 1) * 3 + 1  # dt = 0
                xv = xe[:, base + df * TP: base + df * TP + F * TP]
                wcol = wd[:, blk * 9 + ti: blk * 9 + ti + 1]
                if i == 0:
                    nc.vector.tensor_scalar(dacc[:], xv, wcol, None, OP.mult)
                else:
                    nc.vector.scalar_tensor_tensor(dacc[:], xv, wcol, dacc[:],
                                                   OP.mult, OP.add)

            # diag lhsT tiles for the 6 PE taps (dt = +-1)
            PET = [(df, dt) for df in (-1, 0, 1) for dt in (-1, 1)]
            dg = work.tile([CTX, 6 * CTX], BF16, tag="dg")
            for i, (df, dt) in enumerate(PET):
                ti = (df + 1) * 3 + (dt + 1)
                nc.vector.tensor_scalar(
                    dg[:, i * CTX:(i + 1) * CTX], w_idb[0:CTX, 0:CTX],
                    wd[:, blk * 9 + ti: blk * 9 + ti + 1], None, OP.mult)

            z = zpool.tile([CTX, FT], BF16, tag=f"z{blk}")
            dacc3 = dacc.rearrange("p (f t) -> p f t", t=TP)
            GF = 4  # f-rows per psum group (1 bank)
            for g in range(F // GF):
                ps_d = ps_mm.tile([CTX, GF * T], F32, tag="psd")
                for i, (df, dt) in enumerate(PET):
                    rv = xe3[:, 1 + g * GF + df: 1 + g * GF + df + GF,
                             TOFF + dt: TOFF + dt + T]
                    nc.tensor.matmul(ps_d[:], dg[:, i * CTX:(i + 1) * CTX], rv,
                                     start=(i == 0), stop=False)
                nc.tensor.matmul(ps_d[:], w_idb[0:CTX, 0:CTX],
                                 dacc3[:, g * GF:(g + 1) * GF, TOFF:TOFF + T],
                                 start=False, stop=True)
                y1 = work.tile([CTX, GF * T], BF16, tag="y1")
                y2 = work.tile([CTX, GF * T], BF16, tag="y2")
                nc.scalar.activation(y1[:], ps_d[:], AF.Identity,
                                     bias=cj[2][:, blk * S + s: blk * S + s + 1],
                                     scale=cj[0][:, blk * S + s: blk * S + s + 1])
                nc.scalar.activation(y2[:], ps_d[:], AF.Identity,
                                     bias=cj[3][:, blk * S + s: blk * S + s + 1],
                                     scale=cj[1][:, blk * S + s: blk * S + s + 1])
                zsl = z[:, g * GF * T:(g + 1) * GF * T]
                nc.vector.tensor_tensor(zsl, y1[:], y2[:], OP.max)
                z3 = zsl.rearrange("p (f t) -> p f t", t=T)
                gf_v = sigf[:, (blk * S + s) * F + g * GF:
                            (blk * S + s) * F + (g + 1) * GF] \
                    .unsqueeze(2).broadcast_to((CTX, GF, T))
                nc.vector.tensor_tensor(z3[:], z3[:], gf_v, OP.mult)
                gt_v = sigt[:, (blk * S + s) * T:(blk * S + s + 1) * T] \
                    .unsqueeze(1).broadcast_to((CTX, GF, T))
                nc.gpsimd.tensor_tensor(z3[:], z3[:], gt_v, OP.mult)
            zs.append(z)

        for ch in range(NCH):
            ps_p = ps_mm.tile([COUT, CHW], F32, tag="psp")
            for blk in range(NBLK):
                nc.tensor.matmul(ps_p[:], wpb[:, blk * COUT:(blk + 1) * COUT],
                                 zs[blk][:, ch * CHW:(ch + 1) * CHW],
                                 start=(blk == 0), stop=(blk == NBLK - 1))
            tpo = work.tile([COUT, CHW], F32, tag="tpo")
            nc.scalar.activation(tpo[:], ps_p[:], AF.Identity,
                                 bias=w_pbias[:COUT, 0:1], scale=1.0)
            xr = xpool.tile([CIN, CHW], F32, tag="x0s")
            nc.sync.dma_start(xr[:], xs[s][:, ch * CHW:(ch + 1) * CHW])
            outs = work.tile([COUT, CHW], F32, tag="outs")
            nc.gpsimd.tensor_add(outs[:], tpo[:], xr[:])
            nc.sync.dma_start(
                yout[s][:, ch * CHW:(ch + 1) * CHW], outs[:])


def _host_prep(inputs):
    """Precompute packed/folded weight arrays (numpy, O(weights))."""
    p = {k: np.asarray(v, dtype=np.float32) for k, v in inputs.items()}
    inv_j = p["cg_joint_gamma"] / np.sqrt(p["cg_joint_var"] + EPS)
    sh_j = p["cg_joint_beta"] - p["cg_joint_mean"] * inv_j
    jlf = (p["cg_joint_w"].T * inv_j[None, :]) / T
    jlt = (p["cg_joint_w"].T * inv_j[None, :]) / F
    jb3 = (sh_j + 3.0)[:, None]

    cvf = np.ascontiguousarray(p["cg_convf_w"].T)
    cvt = np.ascontiguousarray(p["cg_convt_w"].T)
    bfb = np.ascontiguousarray(p["cg_convf_b"].reshape(NBLK, CTX).T)
    btb = np.ascontiguousarray(p["cg_convt_b"].reshape(NBLK, CTX).T)

    sc = 1.0 / ((F + T) * TEMP)
    arw0 = np.concatenate([p["exp_res_w"], p["dep_res_w"], p["proj_res_w"]], 0).T * sc
    arb0 = np.concatenate([p["exp_res_b"], p["dep_res_b"], p["proj_res_b"]]) / TEMP
    arw = np.ascontiguousarray(np.vstack([arw0, arb0[None, :]]))

    drw_r = p["dr_w"].reshape(CEXP, 2 * M, CTX).transpose(1, 0, 2)
    drw0 = drw_r.reshape(2 * M * CEXP, CTX).T / (F + T)
    drb_r = p["dr_b"].reshape(CEXP, 2 * M).T.reshape(-1)
    drw = np.ascontiguousarray(np.vstack([drw0, drb_r[None, :]]))

    inv_e = p["exp_bn_gamma"] / np.sqrt(p["exp_bn_var"] + EPS)
    sh_e = p["exp_bn_beta"] - p["exp_bn_mean"] * inv_e
    ew = (p["exp_weight"] * inv_e[None, :, None]).transpose(0, 2, 1)  # [K,80,480]
    expw = np.ascontiguousarray(ew.transpose(1, 0, 2).reshape(CIN, K * CEXP))
    eb3 = np.ascontiguousarray((sh_e + 3.0).reshape(NBLK, CTX).T)

    inv_d = p["dep_bn_gamma"] / np.sqrt(p["dep_bn_var"] + EPS)
    sh_d = p["dep_bn_beta"] - p["dep_bn_mean"] * inv_d
    dw = (p["dep_weight"] * inv_d[None, :, None, None]).reshape(K, CEXP, 9)
    dw_b = dw.reshape(K, NBLK, CTX, 9).transpose(2, 1, 0, 3)
    depw = np.ascontiguousarray(dw_b.reshape(CTX, NBLK * K * 9))
    bnbd = np.ascontiguousarray(sh_d.reshape(NBLK, CTX).T)

    inv_p = p["proj_bn_gamma"] / np.sqrt(p["proj_bn_var"] + EPS)
    sh_p = p["proj_bn_beta"] - p["proj_bn_mean"] * inv_p
    pw = p["proj_weight"] * inv_p[None, :, None]        # [K, 80, 480]
    pw_b = pw.transpose(2, 0, 1).reshape(NBLK, CTX, K, COUT).transpose(1, 0, 2, 3)
    projw = np.ascontiguousarray(pw_b.reshape(CTX, NBLK * K * COUT))
    pbias = sh_p[:, None]

    identf = np.eye(128, dtype=np.float32)
    return dict(jlf=jlf, jlt=jlt, jb3=jb3, cvf=cvf, cvt=cvt, bfb=bfb, btb=btb,
                arw=arw, drw=drw, expw=expw, eb3=eb3,
                depw=depw, bnbd=bnbd, projw=projw, pbias=pbias, identf=identf,
                identb=np.eye(128).astype(ml_dtypes.bfloat16),
                ones1=np.ones((1, S), np.float32))


_BUILT = {}


def _build():
    if "nc" in _BUILT:
        return _BUILT["nc"]
    nc = bacc.Bacc("TRN2", target_bir_lowering=False, debug=False,
                   num_devices=NCORES)
    d = lambda n, s: nc.dram_tensor(n, list(s), F32, kind="ExternalInput").ap()
    io = [
        d("xs", (S, CIN, FT)),
        d("jlf", (CIN, CTX)), d("jlt", (CIN, CTX)), d("jb3", (CTX, 1)),
        d("cvf", (CTX, CEXP)), d("cvt", (CTX, CEXP)),
        d("bfb", (CTX, NBLK)), d("btb", (CTX, NBLK)),
        d("arw", (CTX + 1, 3 * K)),
        d("drw", (CTX + 1, 2 * M * CEXP)),
        d("expw", (CIN, K * CEXP)), d("eb3", (CTX, NBLK)),
        d("depw", (CTX, NBLK * K * 9)), d("bnbd", (CTX, NBLK)),
        d("projw", (CTX, NBLK * K * COUT)), d("pbias", (COUT, 1)),
        d("identf", (128, 128)),
        nc.dram_tensor("identb", [128, 128], BF16, kind="ExternalInput").ap(),
        d("ones1", (1, S)),
        nc.dram_tensor("y", [S, COUT, FT], F32, kind="ExternalOutput").ap(),
    ]
    from contextlib import ExitStack
    with tile.TileContext(nc) as tc:
        with ExitStack() as es:
            _emit(tc, io, es)
    nc.compile()
    _BUILT["nc"] = nc
    return nc


def kernel(**inputs):
    nc = _build()
    host = _host_prep(inputs)
    x = np.asarray(inputs["x"], dtype=np.float32).reshape(B, CIN, FT)
    in_maps = []
    for c in range(NCORES):
        m = {"xs": np.ascontiguousarray(x[c * S:(c + 1) * S])}
        m.update(host)
        in_maps.append(m)
    res = run_bass_kernel_spmd(nc, in_maps, list(range(NCORES)))
    out = np.concatenate([res.results[c]["y"] for c in range(NCORES)], axis=0)
    return out.reshape(B, COUT, F, T)


if __name__ == "__main__":
    import reference as ref
    inp = {k: np.asarray(v) for k, v in ref.setup_inputs().items()}
    got = kernel(**inp)
    from np_ref import forward_np
    exp = forward_np(inp)
    rel = np.abs(got - exp).max() / np.abs(exp).max()
    print("rel err vs np_ref:", rel)


# revision 19
# speedup vs baseline: 1.0873x; 1.0873x over previous
"""Trainium2 Bass kernel for DY_Block (EfficientAT DyMN dynamic block).

Data-parallel over batch B=32 across 8 NeuronCores (4 samples/core); all
intermediates stay on-chip between input load and output store.

Per core:
  Phase A (batched over the 4 local samples):
    means -> ContextGen joint conv (PE; BN + mean-div folded into lhsT) ->
    hswish -> g_c -> routing softmax (PE/ACT/DVE) -> att transposed (PE) and
    partition-broadcast (GPSIMD) -> DyReLU coefs (PE matmul + ACT sigmoid,
    transposed to channel-partition tiles) -> CoordAtt gates (PE + ACT), bf16.
  Phase B per sample:
    mix expert weights (DVE STT chains) -> expansion conv (PE bf16) ->
    BN+hswish (ACT relu + DVE) into padded [c,(f,t)] layout -> depthwise 3x3
    as 9-tap DVE STT chain (shifts = AP offsets) -> DyReLU (ACT FMA x2 + DVE
    max) -> CoordAtt gating (DVE, broadcast views) -> projection conv (PE,
    accumulating) -> BN bias (ACT) + residual (DVE) -> DMA out.
"""
import zlib

import ml_dtypes
import numpy as np

import concourse.bass as bass
import concourse.bacc as bacc
import concourse.tile as tile
from concourse import mybir
from concourse.bass_utils import run_bass_kernel_spmd

F32 = mybir.dt.float32
BF16 = mybir.dt.bfloat16
AX = mybir.AxisListType
OP = mybir.AluOpType
AF = mybir.ActivationFunctionType

B, CIN, CEXP, COUT, F, T = 32, 80, 480, 80, 32, 125
CTX, K, M = 120, 4, 2
TEMP = 30.0
EPS = 1e-3
NCORES = 8
S = B // NCORES          # samples per core
NBLK = CEXP // CTX       # 4 channel blocks of 120
FT = F * T               # 4000
TP = 128                 # padded row stride for depthwise layout
FP = F + 2               # padded f rows
XOFF = 4                 # even lead offset so tap views never start below 0
NPAD = XOFF + FP * TP + 4  # padded per-block free size (+ slack for +1,+1 tap)
TOFF = 2                 # t offset inside padded row (even, for bf16 align)
NCH = 8                  # N-chunks per (blk) matmul pass
# Source-content tag: changes the HLO signature whenever this file changes so
# the neuronx compile cache (which keys on HLO alone) cannot serve a stale NEFF.
_VTAG = (zlib.crc32(open(__file__, 'rb').read()) % 997) + 2
CHW = FT // NCH          # 500 cols per chunk
CHF = F // NCH           # 4 f rows per chunk


def _emit(tc, io, ctx):
    nc = tc.nc

    (xs, jlf, jlt, jb3, cvf, cvt, bfb, btb, arw, drw, expw, eb3,
     depw, bnbd, projw, pbias, identf, vtag, identb, ones1, yout) = io

    wpool = ctx.enter_context(tc.tile_pool(name="weights", bufs=1))
    ctx_pool = ctx.enter_context(tc.tile_pool(name="ctx", bufs=1))
    ps_mm = ctx.enter_context(tc.tile_pool(name="ps_mm", bufs=2, space="PSUM"))
    ps_small = ps_mm
    work = ctx.enter_context(tc.tile_pool(name="work", bufs=2))
    zpool = ctx.enter_context(tc.tile_pool(name="zpool", bufs=1))
    xpool = ctx.enter_context(tc.tile_pool(name="xpool", bufs=2))

    # ---------- persistent weights ----------
    def wtile(ap, tag):
        t = wpool.tile(list(ap.shape), ap.dtype, tag=tag)
        nc.sync.dma_start(t[:], ap)
        return t

    w_jlf = wtile(jlf, "jlf")          # [80,120]
    w_jlt = wtile(jlt, "jlt")          # [80,120]
    w_jb3 = wtile(jb3, "jb3")          # [120,1]
    w_cvf = wtile(cvf, "cvf")          # [120,480]
    w_cvt = wtile(cvt, "cvt")          # [120,480]
    w_bfb = wtile(bfb, "bfb")          # [120,4]
    w_btb = wtile(btb, "btb")          # [120,4]
    w_arw = wtile(arw, "arw")          # [121,12] (last row = bias)
    w_drw = wtile(drw, "drw")          # [121,1920] (last row = bias)
    w_expw = wtile(expw, "expw")       # [80,1920]
    w_eb3 = wtile(eb3, "eb3")          # [120,4]
    w_depw = wtile(depw, "depw")       # [120,144]
    w_bnbd = wtile(bnbd, "bnbd")       # [120,4]
    w_projw = wtile(projw, "projw")    # [120,1280]
    w_pbias = wtile(pbias, "pbias")    # [80,1]
    w_idf = wtile(identf, "idf")       # [128,128] f32 identity
    w_idb = wtile(identb, "idb")       # [128,128] bf16 identity

    # ---------- Phase A : batched context ----------
    ga_in = ctx_pool.tile([CIN, S * (F + T)], F32)
    for s in range(S):
        for ch in range(NCH):
            x0a = xpool.tile([CIN, CHW], F32, tag="x0s")
            nc.sync.dma_start(x0a[:], xs[s][:, ch * CHW:(ch + 1) * CHW])
            nc.vector.tensor_reduce(
                ga_in[:, s * F + ch * CHF:(s * F) + (ch + 1) * CHF],
                x0a.rearrange("p (f t) -> p f t", f=CHF), AX.X, OP.add)
            ctc = ga_in[:, S * F + s * T: S * F + (s + 1) * T]
            if ch == 0:
                nc.vector.tensor_reduce(
                    ctc, x0a.rearrange("p (f t) -> p t f", f=CHF), AX.X, OP.add)
            else:
                ctp = work.tile([CIN, T], F32, tag="ctp")
                nc.vector.tensor_reduce(
                    ctp[:], x0a.rearrange("p (f t) -> p t f", f=CHF), AX.X, OP.add)
                nc.vector.tensor_add(ctc, ctc, ctp[:])

    ps_g1 = ps_small.tile([CTX, S * F], F32, tag="pse")
    nc.tensor.matmul(ps_g1[:], w_jlf[:], ga_in[:, 0:S * F], start=True, stop=True)
    ps_g2 = ps_small.tile([CTX, S * T], F32, tag="pse")
    nc.tensor.matmul(ps_g2[:], w_jlt[:], ga_in[:, S * F:], start=True, stop=True)

    r_g = ctx_pool.tile([CTX, S * (F + T)], F32)
    nc.scalar.activation(r_g[:, 0:S * F], ps_g1[:], AF.Relu, bias=w_jb3[:, 0:1], scale=1.0)
    nc.scalar.activation(r_g[:, S * F:], ps_g2[:], AF.Relu, bias=w_jb3[:, 0:1], scale=1.0)
    v_g = ctx_pool.tile([CTX, S * (F + T)], F32)
    nc.vector.tensor_scalar(v_g[:], r_g[:], 6.0, 1.0 / 6.0, OP.min, OP.mult)
    gc_t = r_g  # in-place: (r-3)*v overwrites r
    nc.vector.scalar_tensor_tensor(gc_t[:], r_g[:], -3.0, v_g[:], OP.add, OP.mult)

    g_c = ctx_pool.tile([CTX + 1, S], F32)
    tmp_r = ctx_pool.tile([CTX, S], F32)
    nc.vector.tensor_reduce(
        g_c[0:CTX, :], gc_t[:, 0:S * F].rearrange("p (s f) -> p s f", s=S),
        AX.X, OP.add)
    nc.vector.tensor_reduce(
        tmp_r[:], gc_t[:, S * F:].rearrange("p (s t) -> p s t", s=S), AX.X, OP.add)
    nc.vector.tensor_add(g_c[0:CTX, :], g_c[0:CTX, :], tmp_r[:])
    nc.sync.dma_start(g_c[CTX:CTX + 1, :], ones1)

    # routing attention
    ps_a = ps_small.tile([S, 3 * K], F32, tag="pse")
    nc.tensor.matmul(ps_a[:], g_c[:], w_arw[:], start=True, stop=True)
    ex_t = ctx_pool.tile([S, 3 * K], F32)
    nc.scalar.activation(ex_t[:], ps_a[:], AF.Exp)
    s3 = ctx_pool.tile([S, 3], F32)
    nc.vector.tensor_reduce(
        s3[:], ex_t.rearrange("p (j k) -> p j k", j=3), AX.X, OP.add)
    rec3 = ctx_pool.tile([S, 3], F32)
    nc.vector.reciprocal(rec3[:], s3[:])
    attn = ctx_pool.tile([S, 3 * K], F32)
    for j in range(3):
        nc.vector.tensor_scalar(
            attn[:, j * K:(j + 1) * K], ex_t[:, j * K:(j + 1) * K],
            rec3[:, j:j + 1], None, OP.mult)
    # att48[s, (jk, s')] = attn[s, jk] * I[s, s']; ones-matmul broadcasts to
    # all partitions: att_b[p, (jk, s)] = attn[s, jk]
    att48 = ctx_pool.tile([S, 3 * K * S], F32)
    nc.vector.tensor_tensor(
        att48.rearrange("p (jk s) -> p jk s", s=S),
        attn.unsqueeze(2).broadcast_to((S, 3 * K, S)),
        w_idf[0:S, 0:S].unsqueeze(1).broadcast_to((S, 3 * K, S)),
        OP.mult)
    onesS = ctx_pool.tile([S, CTX], F32)
    nc.vector.memset(onesS[:], 1.0)
    ps_ab = ps_small.tile([CTX, 3 * K * S], F32, tag="pse")
    nc.tensor.matmul(ps_ab[:], onesS[:], att48[:], start=True, stop=True)
    att_b = ctx_pool.tile([CTX, 3 * K * S], F32)
    nc.scalar.copy(att_b[:], ps_ab[:])

    # DyReLU coefficients
    coefs = ctx_pool.tile([S, 2 * M * CEXP], F32)
    for j in range(4):
        ps_th = ps_small.tile([S, CEXP], F32, tag="pse")
        nc.tensor.matmul(ps_th[:], g_c[:], w_drw[:, j * CEXP:(j + 1) * CEXP],
                         start=True, stop=True)
        nc.scalar.activation(coefs[:, j * CEXP:(j + 1) * CEXP], ps_th[:], AF.Sigmoid)
    # theta' = 2*sig - 1; a1 = theta'+1 = 2*sig; a2 = 2*sig-1; b = sig-0.5
    for j, (sc, of) in enumerate([(2.0, 0.0), (2.0, -1.0), (1.0, -0.5), (1.0, -0.5)]):
        nc.vector.tensor_scalar(coefs[:, j * CEXP:(j + 1) * CEXP],
                                coefs[:, j * CEXP:(j + 1) * CEXP],
                                sc, of, OP.mult, OP.add)
    cj = []
    for j in range(4):
        cj_t = ctx_pool.tile([CTX, NBLK * S], F32, tag=f"cj{j}")
        for blk in range(NBLK):
            ps_c = ps_small.tile([CTX, S], F32, tag="pse")
            nc.tensor.transpose(
                ps_c[:], coefs[:, j * CEXP + blk * CTX: j * CEXP + (blk + 1) * CTX],
                w_idf[0:S, 0:S])
            nc.scalar.copy(cj_t[:, blk * S:(blk + 1) * S], ps_c[:])
        cj.append(cj_t)
    for i in range(2):  # fold dep-BN bias: b'_i = a_i*bnb + b_i
        for blk in range(NBLK):
            sl = slice(blk * S, (blk + 1) * S)
            nc.vector.scalar_tensor_tensor(
                cj[2 + i][:, sl], cj[i][:, sl], w_bnbd[:, blk:blk + 1],
                cj[2 + i][:, sl], OP.mult, OP.add)

    # CoordAtt gates (bf16)
    sigf = ctx_pool.tile([CTX, NBLK * S * F], BF16)
    sigt = ctx_pool.tile([CTX, NBLK * S * T], BF16)
    for blk in range(NBLK):
        ps_f = ps_small.tile([CTX, S * F], F32, tag="pse")
        nc.tensor.matmul(ps_f[:], w_cvf[:, blk * CTX:(blk + 1) * CTX],
                         gc_t[:, 0:S * F], start=True, stop=True)
        nc.scalar.activation(sigf[:, blk * S * F:(blk + 1) * S * F], ps_f[:],
                             AF.Sigmoid, bias=w_bfb[:, blk:blk + 1], scale=1.0)
        ps_t2 = ps_small.tile([CTX, S * T], F32, tag="pse")
        nc.tensor.matmul(ps_t2[:], w_cvt[:, blk * CTX:(blk + 1) * CTX],
                         gc_t[:, S * F:], start=True, stop=True)
        nc.scalar.activation(sigt[:, blk * S * T:(blk + 1) * S * T], ps_t2[:],
                             AF.Sigmoid, bias=w_btb[:, blk:blk + 1], scale=1.0)

    # ---------- Phase B : per-sample heavy pipeline ----------

    for s in range(S):
        x0b = xpool.tile([CIN, FT], BF16, tag="x0b", bufs=1)
        for ch in range(NCH):
            x0a = xpool.tile([CIN, CHW], F32, tag="x0s")
            nc.sync.dma_start(x0a[:], xs[s][:, ch * CHW:(ch + 1) * CHW])
            nc.vector.tensor_copy(x0b[:, ch * CHW:(ch + 1) * CHW], x0a[:])

        def mix(dst, src_sl, jr, parts):
            for k in range(K):
                c0 = (jr * K + k) * S + s
                a_col = att_b[0:parts, c0:c0 + 1]
                if k == 0:
                    nc.vector.tensor_scalar(dst, src_sl(k), a_col, None, OP.mult)
                else:
                    nc.vector.scalar_tensor_tensor(dst, src_sl(k), a_col, dst,
                                                   OP.mult, OP.add)

        we = work.tile([CIN, CEXP], F32, tag="we")
        mix(we[:], lambda k: w_expw[:, k * CEXP:(k + 1) * CEXP], 0, CIN)
        web = work.tile([CIN, CEXP], BF16, tag="web")
        nc.vector.tensor_copy(web[:], we[:])

        wd = work.tile([CTX, NBLK * 9], F32, tag="wd")
        for blk in range(NBLK):
            mix(wd[:, blk * 9:(blk + 1) * 9],
                lambda k: w_depw[:, (blk * K + k) * 9:(blk * K + k + 1) * 9], 1, CTX)

        wp = work.tile([CTX, NBLK * COUT], F32, tag="wp")
        for blk in range(NBLK):
            mix(wp[:, blk * COUT:(blk + 1) * COUT],
                lambda k: w_projw[:, (blk * K + k) * COUT:(blk * K + k + 1) * COUT],
                2, CTX)
        wpb = work.tile([CTX, NBLK * COUT], BF16, tag="wpb")
        nc.vector.tensor_copy(wpb[:], wp[:])

        zs = []
        for blk in range(NBLK):
            r_blk = work.tile([CTX, FT], BF16, tag="sa")
            for ch in range(NCH):
                ps_e = ps_mm.tile([CTX, CHW], F32, tag="pse")
                nc.tensor.matmul(ps_e[:], web[:, blk * CTX:(blk + 1) * CTX],
                                 x0b[:, ch * CHW:(ch + 1) * CHW],
                                 start=True, stop=True)
                nc.scalar.activation(r_blk[:, ch * CHW:(ch + 1) * CHW], ps_e[:],
                                     AF.Relu, bias=w_eb3[:, blk:blk + 1], scale=1.0)
            v_blk = work.tile([CTX, FT], BF16, tag="sb")
            nc.vector.tensor_scalar(v_blk[:], r_blk[:], 6.0, 1.0 / 6.0, OP.min, OP.mult)
            xe = work.tile([CTX, NPAD], BF16, tag="xe")
            # zero only the pad regions (lead row, tail row, t-pad columns)
            nc.vector.memset(xe[:, 0:XOFF + TP], 0.0)
            nc.vector.memset(xe[:, XOFF + (F + 1) * TP:NPAD], 0.0)
            xep = xe[:, XOFF:XOFF + FP * TP]
            nc.vector.memset(
                xep.rearrange("p (f t) -> p f t", t=TP)[:, 1:1 + F, 0:TOFF], 0.0)
            nc.vector.memset(
                xep.rearrange("p (f t) -> p f t", t=TP)[:, 1:1 + F, TOFF + T:TP], 0.0)
            xe3 = xep.rearrange("p (f t) -> p f t", t=TP)
            nc.vector.scalar_tensor_tensor(
                xe3[:, 1:1 + F, TOFF:TOFF + T],
                r_blk.rearrange("p (f t) -> p f t", t=T), -3.0,
                v_blk.rearrange("p (f t) -> p f t", t=T), OP.add, OP.mult)

            # DVE partial: the 3 aligned taps (dt=0)
            dacc = work.tile([CTX, F * TP], BF16, tag="dacc")
            base = XOFF + TP
            for i, df in enumerate((-1, 0, 1)):
                ti = (df + 1) * 3 + 1  # dt = 0
                xv = xe[:, base + df * TP: base + df * TP + F * TP]
                wcol = wd[:, blk * 9 + ti: blk * 9 + ti + 1]
                if i == 0:
                    nc.vector.tensor_scalar(dacc[:], xv, wcol, None, OP.mult)
                else:
                    nc.vector.scalar_tensor_tensor(dacc[:], xv, wcol, dacc[:],
                                                   OP.mult, OP.add)

            # diag lhsT tiles for the 6 PE taps (dt = +-1)
            PET = [(df, dt) for df in (-1, 0, 1) for dt in (-1, 1)]
            dg = work.tile([CTX, 6 * CTX], BF16, tag="dg")
            for i, (df, dt) in enumerate(PET):
                ti = (df + 1) * 3 + (dt + 1)
                nc.vector.tensor_scalar(
                    dg[:, i * CTX:(i + 1) * CTX], w_idb[0:CTX, 0:CTX],
                    wd[:, blk * 9 + ti: blk * 9 + ti + 1], None, OP.mult)

            z = zpool.tile([CTX, FT], BF16, tag=f"z{blk}")
            dacc3 = dacc.rearrange("p (f t) -> p f t", t=TP)
            GF = 4  # f-rows per psum group (1 bank)
            for g in range(F // GF):
                ps_d = ps_mm.tile([CTX, GF * T], F32, tag="psd")
                for i, (df, dt) in enumerate(PET):
                    rv = xe3[:, 1 + g * GF + df: 1 + g * GF + df + GF,
                             TOFF + dt: TOFF + dt + T]
                    nc.tensor.matmul(ps_d[:], dg[:, i * CTX:(i + 1) * CTX], rv,
                                     start=(i == 0), stop=False)
                nc.tensor.matmul(ps_d[:], w_idb[0:CTX, 0:CTX],
                                 dacc3[:, g * GF:(g + 1) * GF, TOFF:TOFF + T],
                                 start=False, stop=True)
                y1 = work.tile([CTX, GF * T], BF16, tag="y1")
                y2 = work.tile([CTX, GF * T], BF16, tag="y2")
                nc.scalar.activation(y1[:], ps_d[:], AF.Identity,
                                     bias=cj[2][:, blk * S + s: blk * S + s + 1],
                                     scale=cj[0][:, blk * S + s: blk * S + s + 1])
                nc.scalar.activation(y2[:], ps_d[:], AF.Identity,
                                     bias=cj[3][:, blk * S + s: blk * S + s + 1],
                                     scale=cj[1][:, blk * S + s: blk * S + s + 1])
                zsl = z[:, g * GF * T:(g + 1) * GF * T]
                nc.vector.tensor_tensor(zsl, y1[:], y2[:], OP.max)
                z3 = zsl.rearrange("p (f t) -> p f t", t=T)
                gf_v = sigf[:, (blk * S + s) * F + g * GF:
                            (blk * S + s) * F + (g + 1) * GF] \
                    .unsqueeze(2).broadcast_to((CTX, GF, T))
                nc.vector.tensor_tensor(z3[:], z3[:], gf_v, OP.mult)
                gt_v = sigt[:, (blk * S + s) * T:(blk * S + s + 1) * T] \
                    .unsqueeze(1).broadcast_to((CTX, GF, T))
                nc.gpsimd.tensor_tensor(z3[:], z3[:], gt_v, OP.mult)
            zs.append(z)

        for ch in range(NCH):
            ps_p = ps_mm.tile([COUT, CHW], F32, tag="psp")
            for blk in range(NBLK):
                nc.tensor.matmul(ps_p[:], wpb[:, blk * COUT:(blk + 1) * COUT],
                                 zs[blk][:, ch * CHW:(ch + 1) * CHW],
                                 start=(blk == 0), stop=(blk == NBLK - 1))
            tpo = work.tile([COUT, CHW], F32, tag="tpo")
            nc.scalar.activation(tpo[:], ps_p[:], AF.Identity,
                                 bias=w_pbias[:COUT, 0:1], scale=1.0)
            xr = xpool.tile([CIN, CHW], F32, tag="x0s")
            nc.sync.dma_start(xr[:], xs[s][:, ch * CHW:(ch + 1) * CHW])
            outs = work.tile([COUT, CHW], F32, tag="outs")
            nc.gpsimd.tensor_add(outs[:], tpo[:], xr[:])
            nc.sync.dma_start(
                yout[s][:, ch * CHW:(ch + 1) * CHW], outs[:])


def _host_prep(inputs):
    """Precompute packed/folded weight arrays (numpy, O(weights))."""
    p = {k: np.asarray(v, dtype=np.float32) for k, v in inputs.items()}
    inv_j = p["cg_joint_gamma"] / np.sqrt(p["cg_joint_var"] + EPS)
    sh_j = p["cg_joint_beta"] - p["cg_joint_mean"] * inv_j
    jlf = (p["cg_joint_w"].T * inv_j[None, :]) / T
    jlt = (p["cg_joint_w"].T * inv_j[None, :]) / F
    jb3 = (sh_j + 3.0)[:, None]

    cvf = np.ascontiguousarray(p["cg_convf_w"].T)
    cvt = np.ascontiguousarray(p["cg_convt_w"].T)
    bfb = np.ascontiguousarray(p["cg_convf_b"].reshape(NBLK, CTX).T)
    btb = np.ascontiguousarray(p["cg_convt_b"].reshape(NBLK, CTX).T)

    sc = 1.0 / ((F + T) * TEMP)
    arw0 = np.concatenate([p["exp_res_w"], p["dep_res_w"], p["proj_res_w"]], 0).T * sc
    arb0 = np.concatenate([p["exp_res_b"], p["dep_res_b"], p["proj_res_b"]]) / TEMP
    arw = np.ascontiguousarray(np.vstack([arw0, arb0[None, :]]))

    drw_r = p["dr_w"].reshape(CEXP, 2 * M, CTX).transpose(1, 0, 2)
    drw0 = drw_r.reshape(2 * M * CEXP, CTX).T / (F + T)
    drb_r = p["dr_b"].reshape(CEXP, 2 * M).T.reshape(-1)
    drw = np.ascontiguousarray(np.vstack([drw0, drb_r[None, :]]))

    inv_e = p["exp_bn_gamma"] / np.sqrt(p["exp_bn_var"] + EPS)
    sh_e = p["exp_bn_beta"] - p["exp_bn_mean"] * inv_e
    ew = (p["exp_weight"] * inv_e[None, :, None]).transpose(0, 2, 1)  # [K,80,480]
    expw = np.ascontiguousarray(ew.transpose(1, 0, 2).reshape(CIN, K * CEXP))
    eb3 = np.ascontiguousarray((sh_e + 3.0).reshape(NBLK, CTX).T)

    inv_d = p["dep_bn_gamma"] / np.sqrt(p["dep_bn_var"] + EPS)
    sh_d = p["dep_bn_beta"] - p["dep_bn_mean"] * inv_d
    dw = (p["dep_weight"] * inv_d[None, :, None, None]).reshape(K, CEXP, 9)
    dw_b = dw.reshape(K, NBLK, CTX, 9).transpose(2, 1, 0, 3)
    depw = np.ascontiguousarray(dw_b.reshape(CTX, NBLK * K * 9))
    bnbd = np.ascontiguousarray(sh_d.reshape(NBLK, CTX).T)

    inv_p = p["proj_bn_gamma"] / np.sqrt(p["proj_bn_var"] + EPS)
    sh_p = p["proj_bn_beta"] - p["proj_bn_mean"] * inv_p
    pw = p["proj_weight"] * inv_p[None, :, None]        # [K, 80, 480]
    pw_b = pw.transpose(2, 0, 1).reshape(NBLK, CTX, K, COUT).transpose(1, 0, 2, 3)
    projw = np.ascontiguousarray(pw_b.reshape(CTX, NBLK * K * COUT))
    pbias = sh_p[:, None]

    identf = np.eye(128, dtype=np.float32)
    return dict(jlf=jlf, jlt=jlt, jb3=jb3, cvf=cvf, cvt=cvt, bfb=bfb, btb=btb,
                arw=arw, drw=drw, expw=expw, eb3=eb3,
                depw=depw, bnbd=bnbd, projw=projw, pbias=pbias, identf=identf,
                vtag=np.zeros((1, _VTAG), np.float32),
                identb=np.eye(128).astype(ml_dtypes.bfloat16),
                ones1=np.ones((1, S), np.float32))


_BUILT = {}


def _build():
    if "nc" in _BUILT:
        return _BUILT["nc"]
    nc = bacc.Bacc("TRN2", target_bir_lowering=False, debug=False,
                   num_devices=NCORES)
    d = lambda n, s: nc.dram_tensor(n, list(s), F32, kind="ExternalInput").ap()
    io = [
        d("xs", (S, CIN, FT)),
        d("jlf", (CIN, CTX)), d("jlt", (CIN, CTX)), d("jb3", (CTX, 1)),
        d("cvf", (CTX, CEXP)), d("cvt", (CTX, CEXP)),
        d("bfb", (CTX, NBLK)), d("btb", (CTX, NBLK)),
        d("arw", (CTX + 1, 3 * K)),
        d("drw", (CTX + 1, 2 * M * CEXP)),
        d("expw", (CIN, K * CEXP)), d("eb3", (CTX, NBLK)),
        d("depw", (CTX, NBLK * K * 9)), d("bnbd", (CTX, NBLK)),
        d("projw", (CTX, NBLK * K * COUT)), d("pbias", (COUT, 1)),
        d("identf", (128, 128)), d("vtag", (1, _VTAG)),
        nc.dram_tensor("identb", [128, 128], BF16, kind="ExternalInput").ap(),
        d("ones1", (1, S)),
        nc.dram_tensor("y", [S, COUT, FT], F32, kind="ExternalOutput").ap(),
    ]
    from contextlib import ExitStack
    with tile.TileContext(nc) as tc:
        with ExitStack() as es:
            _emit(tc, io, es)
    nc.compile()
    _BUILT["nc"] = nc
    return nc


def kernel(**inputs):
    nc = _build()
    host = _host_prep(inputs)
    x = np.asarray(inputs["x"], dtype=np.float32).reshape(B, CIN, FT)
    in_maps = []
    for c in range(NCORES):
        m = {"xs": np.ascontiguousarray(x[c * S:(c + 1) * S])}
        m.update(host)
        in_maps.append(m)
    res = run_bass_kernel_spmd(nc, in_maps, list(range(NCORES)))
    out = np.concatenate([res.results[c]["y"] for c in range(NCORES)], axis=0)
    return out.reshape(B, COUT, F, T)


if __name__ == "__main__":
    import reference as ref
    inp = {k: np.asarray(v) for k, v in ref.setup_inputs().items()}
    got = kernel(**inp)
    from np_ref import forward_np
    exp = forward_np(inp)
    rel = np.abs(got - exp).max() / np.abs(exp).max()
    print("rel err vs np_ref:", rel)


# revision 33
# speedup vs baseline: 58.4412x; 53.7491x over previous
"""Trainium2 Bass kernel for DY_Block (EfficientAT DyMN dynamic block).

Data-parallel over batch B=32 across 8 NeuronCores (4 samples/core); all
intermediates stay on-chip between input load and output store.

Per core:
  Phase A (batched over the 4 local samples):
    means -> ContextGen joint conv (PE; BN + mean-div folded into lhsT) ->
    hswish -> g_c -> routing softmax (PE/ACT/DVE) -> att transposed (PE) and
    partition-broadcast (GPSIMD) -> DyReLU coefs (PE matmul + ACT sigmoid,
    transposed to channel-partition tiles) -> CoordAtt gates (PE + ACT), bf16.
  Phase B per sample:
    mix expert weights (DVE STT chains) -> expansion conv (PE bf16) ->
    BN+hswish (ACT relu + DVE) into padded [c,(f,t)] layout -> depthwise 3x3
    as 9-tap DVE STT chain (shifts = AP offsets) -> DyReLU (ACT FMA x2 + DVE
    max) -> CoordAtt gating (DVE, broadcast views) -> projection conv (PE,
    accumulating) -> BN bias (ACT) + residual (DVE) -> DMA out.
"""
import os
import zlib

import ml_dtypes
import numpy as np

import concourse.bass as bass
import concourse.bacc as bacc
import concourse.tile as tile
from concourse import mybir
from concourse.bass_utils import run_bass_kernel_spmd

F32 = mybir.dt.float32
BF16 = mybir.dt.bfloat16
AX = mybir.AxisListType
OP = mybir.AluOpType
AF = mybir.ActivationFunctionType

B, CIN, CEXP, COUT, F, T = 32, 80, 480, 80, 32, 125
CTX, K, M = 120, 4, 2
TEMP = 30.0
EPS = 1e-3
NCORES = 8
S = B // NCORES          # samples per core
NBLK = CEXP // CTX       # 4 channel blocks of 120
FT = F * T               # 4000
TP = 128                 # padded row stride for depthwise layout
FP = F + 2               # padded f rows
XOFF = 4                 # even lead offset so tap views never start below 0
NPAD = XOFF + FP * TP + 4  # padded per-block free size (+ slack for +1,+1 tap)
TOFF = 2                 # t offset inside padded row (even, for bf16 align)
NCH = 8                  # N-chunks per (blk) matmul pass
# Source-content tag: changes the HLO signature whenever this file changes so
# the neuronx compile cache (which keys on HLO alone) cannot serve a stale NEFF.
_VTAG = (zlib.crc32(open(__file__, 'rb').read()) % 997) + 2
CHW = FT // NCH          # 500 cols per chunk
CHF = F // NCH           # 4 f rows per chunk


def _emit(tc, io, ctx):
    nc = tc.nc

    (xs, jlf, jlt, jb3, cvf, cvt, bfb, btb, arw, drw, expw, eb3,
     depw, bnbd, projw, pbias, identf, vtag, identb, ones1, yout) = io

    wpool = ctx.enter_context(tc.tile_pool(name="weights", bufs=1))
    ctx_pool = ctx.enter_context(tc.tile_pool(name="ctx", bufs=1))
    ps_mm = ctx.enter_context(tc.tile_pool(name="ps_mm", bufs=2, space="PSUM"))
    ps_small = ps_mm
    work = ctx.enter_context(tc.tile_pool(name="work", bufs=2))
    zpool = ctx.enter_context(tc.tile_pool(name="zpool", bufs=1))
    xpool = ctx.enter_context(tc.tile_pool(name="xpool", bufs=2))

    # ---------- persistent weights ----------
    def wtile(ap, tag):
        t = wpool.tile(list(ap.shape), ap.dtype, tag=tag)
        nc.sync.dma_start(t[:], ap)
        return t

    w_jlf = wtile(jlf, "jlf")          # [80,120]
    w_jlt = wtile(jlt, "jlt")          # [80,120]
    w_jb3 = wtile(jb3, "jb3")          # [120,1]
    w_cvf = wtile(cvf, "cvf")          # [120,480]
    w_cvt = wtile(cvt, "cvt")          # [120,480]
    w_bfb = wtile(bfb, "bfb")          # [120,4]
    w_btb = wtile(btb, "btb")          # [120,4]
    w_arw = wtile(arw, "arw")          # [121,12] (last row = bias)
    w_drw = wtile(drw, "drw")          # [121,1920] (last row = bias)
    w_expw = wtile(expw, "expw")       # [80,1920]
    w_eb3 = wtile(eb3, "eb3")          # [120,4]
    w_depw = wtile(depw, "depw")       # [120,144]
    w_bnbd = wtile(bnbd, "bnbd")       # [120,4]
    w_projw = wtile(projw, "projw")    # [120,1280]
    w_pbias = wtile(pbias, "pbias")    # [80,1]
    w_idf = wtile(identf, "idf")       # [128,128] f32 identity
    w_idb = wtile(identb, "idb")       # [128,128] bf16 identity

    # ---------- Phase A : batched context ----------
    ga_in = ctx_pool.tile([CIN, S * (F + T)], F32)
    MW, MF = 2 * CHW, 2 * CHF
    for s in range(S):
        for ch in range(NCH // 2):
            x0a = xpool.tile([CIN, MW], F32, tag="x0m")
            nc.sync.dma_start(x0a[:], xs[s][:, ch * MW:(ch + 1) * MW])
            nc.vector.tensor_reduce(
                ga_in[:, s * F + ch * MF:(s * F) + (ch + 1) * MF],
                x0a.rearrange("p (f t) -> p f t", f=MF), AX.X, OP.add)
            ctc = ga_in[:, S * F + s * T: S * F + (s + 1) * T]
            if ch == 0:
                nc.vector.tensor_reduce(
                    ctc, x0a.rearrange("p (f t) -> p t f", f=MF), AX.X, OP.add)
            else:
                ctp = work.tile([CIN, T], F32, tag="ctp")
                nc.vector.tensor_reduce(
                    ctp[:], x0a.rearrange("p (f t) -> p t f", f=MF), AX.X, OP.add)
                nc.vector.tensor_add(ctc, ctc, ctp[:])

    ps_g1 = ps_small.tile([CTX, S * F], F32, tag="pse", bufs=3)
    nc.tensor.matmul(ps_g1[:], w_jlf[:], ga_in[:, 0:S * F], start=True, stop=True)
    ps_g2 = ps_small.tile([CTX, S * T], F32, tag="pse", bufs=3)
    nc.tensor.matmul(ps_g2[:], w_jlt[:], ga_in[:, S * F:], start=True, stop=True)

    r_g = ctx_pool.tile([CTX, S * (F + T)], F32)
    nc.scalar.activation(r_g[:, 0:S * F], ps_g1[:], AF.Relu, bias=w_jb3[:, 0:1], scale=1.0)
    nc.scalar.activation(r_g[:, S * F:], ps_g2[:], AF.Relu, bias=w_jb3[:, 0:1], scale=1.0)
    v_g = ctx_pool.tile([CTX, S * (F + T)], F32)
    nc.vector.tensor_scalar(v_g[:], r_g[:], 6.0, 1.0 / 6.0, OP.min, OP.mult)
    gc_t = r_g  # in-place: (r-3)*v overwrites r
    nc.vector.scalar_tensor_tensor(gc_t[:], r_g[:], -3.0, v_g[:], OP.add, OP.mult)

    g_c = ctx_pool.tile([CTX + 1, S], F32)
    tmp_r = ctx_pool.tile([CTX, S], F32)
    nc.vector.tensor_reduce(
        g_c[0:CTX, :], gc_t[:, 0:S * F].rearrange("p (s f) -> p s f", s=S),
        AX.X, OP.add)
    nc.vector.tensor_reduce(
        tmp_r[:], gc_t[:, S * F:].rearrange("p (s t) -> p s t", s=S), AX.X, OP.add)
    nc.vector.tensor_add(g_c[0:CTX, :], g_c[0:CTX, :], tmp_r[:])
    nc.sync.dma_start(g_c[CTX:CTX + 1, :], ones1)

    # routing attention
    ps_a = ps_small.tile([S, 3 * K], F32, tag="pse", bufs=3)
    nc.tensor.matmul(ps_a[:], g_c[:], w_arw[:], start=True, stop=True)
    ex_t = ctx_pool.tile([S, 3 * K], F32)
    nc.scalar.activation(ex_t[:], ps_a[:], AF.Exp)
    s3 = ctx_pool.tile([S, 3], F32)
    nc.vector.tensor_reduce(
        s3[:], ex_t.rearrange("p (j k) -> p j k", j=3), AX.X, OP.add)
    rec3 = ctx_pool.tile([S, 3], F32)
    nc.vector.reciprocal(rec3[:], s3[:])
    attn = ctx_pool.tile([S, 3 * K], F32)
    for j in range(3):
        nc.vector.tensor_scalar(
            attn[:, j * K:(j + 1) * K], ex_t[:, j * K:(j + 1) * K],
            rec3[:, j:j + 1], None, OP.mult)
    # att48[s, (jk, s')] = attn[s, jk] * I[s, s']; ones-matmul broadcasts to
    # all partitions: att_b[p, (jk, s)] = attn[s, jk]
    att48 = ctx_pool.tile([S, 3 * K * S], F32)
    nc.vector.tensor_tensor(
        att48.rearrange("p (jk s) -> p jk s", s=S),
        attn.unsqueeze(2).broadcast_to((S, 3 * K, S)),
        w_idf[0:S, 0:S].unsqueeze(1).broadcast_to((S, 3 * K, S)),
        OP.mult)
    onesS = ctx_pool.tile([S, CTX], F32)
    nc.vector.memset(onesS[:], 1.0)
    ps_ab = ps_small.tile([CTX, 3 * K * S], F32, tag="pse", bufs=3)
    nc.tensor.matmul(ps_ab[:], onesS[:], att48[:], start=True, stop=True)
    att_b = ctx_pool.tile([CTX, 3 * K * S], F32)
    nc.scalar.copy(att_b[:], ps_ab[:])

    # DyReLU coefficients
    coefs = ctx_pool.tile([S, 2 * M * CEXP], F32)
    for j in range(4):
        ps_th = ps_small.tile([S, CEXP], F32, tag="pse", bufs=3)
        nc.tensor.matmul(ps_th[:], g_c[:], w_drw[:, j * CEXP:(j + 1) * CEXP],
                         start=True, stop=True)
        nc.scalar.activation(coefs[:, j * CEXP:(j + 1) * CEXP], ps_th[:], AF.Sigmoid)
    # theta' = 2*sig - 1; a1 = theta'+1 = 2*sig; a2 = 2*sig-1; b = sig-0.5
    for j, (sc, of) in enumerate([(2.0, 0.0), (2.0, -1.0), (1.0, -0.5), (1.0, -0.5)]):
        nc.vector.tensor_scalar(coefs[:, j * CEXP:(j + 1) * CEXP],
                                coefs[:, j * CEXP:(j + 1) * CEXP],
                                sc, of, OP.mult, OP.add)
    cj = []
    for j in range(4):
        cj_t = ctx_pool.tile([CTX, NBLK * S], F32, tag=f"cj{j}")
        for blk in range(NBLK):
            ps_c = ps_small.tile([CTX, S], F32, tag="pse", bufs=3)
            nc.tensor.transpose(
                ps_c[:], coefs[:, j * CEXP + blk * CTX: j * CEXP + (blk + 1) * CTX],
                w_idf[0:S, 0:S])
            nc.scalar.copy(cj_t[:, blk * S:(blk + 1) * S], ps_c[:])
        cj.append(cj_t)
    for i in range(2):  # fold dep-BN bias: b'_i = a_i*bnb + b_i
        for blk in range(NBLK):
            sl = slice(blk * S, (blk + 1) * S)
            nc.vector.scalar_tensor_tensor(
                cj[2 + i][:, sl], cj[i][:, sl], w_bnbd[:, blk:blk + 1],
                cj[2 + i][:, sl], OP.mult, OP.add)

    # CoordAtt gates (bf16)
    sigf = ctx_pool.tile([CTX, NBLK * S * F], BF16)
    sigt = ctx_pool.tile([CTX, NBLK * S * T], BF16)
    for blk in range(NBLK):
        ps_f = ps_small.tile([CTX, S * F], F32, tag="pse", bufs=3)
        nc.tensor.matmul(ps_f[:], w_cvf[:, blk * CTX:(blk + 1) * CTX],
                         gc_t[:, 0:S * F], start=True, stop=True)
        nc.scalar.activation(sigf[:, blk * S * F:(blk + 1) * S * F], ps_f[:],
                             AF.Sigmoid, bias=w_bfb[:, blk:blk + 1], scale=1.0)
        ps_t2 = ps_small.tile([CTX, S * T], F32, tag="pse", bufs=3)
        nc.tensor.matmul(ps_t2[:], w_cvt[:, blk * CTX:(blk + 1) * CTX],
                         gc_t[:, S * F:], start=True, stop=True)
        nc.scalar.activation(sigt[:, blk * S * T:(blk + 1) * S * T], ps_t2[:],
                             AF.Sigmoid, bias=w_btb[:, blk:blk + 1], scale=1.0)

    # ---------- Phase B : per-sample heavy pipeline ----------

    for s in range(S):
        x0b = xpool.tile([CIN, FT], BF16, tag="x0b", bufs=2)
        for ch in range(NCH):
            x0a = xpool.tile([CIN, CHW], F32, tag="x0s")
            nc.sync.dma_start(x0a[:], xs[s][:, ch * CHW:(ch + 1) * CHW])
            nc.vector.tensor_copy(x0b[:, ch * CHW:(ch + 1) * CHW], x0a[:])

        def mix(dst, src_sl, jr, parts):
            for k in range(K):
                c0 = (jr * K + k) * S + s
                a_col = att_b[0:parts, c0:c0 + 1]
                if k == 0:
                    nc.vector.tensor_scalar(dst, src_sl(k), a_col, None, OP.mult)
                else:
                    nc.vector.scalar_tensor_tensor(dst, src_sl(k), a_col, dst,
                                                   OP.mult, OP.add)

        we = work.tile([CIN, CEXP], F32, tag="we")
        mix(we[:], lambda k: w_expw[:, k * CEXP:(k + 1) * CEXP], 0, CIN)
        web = work.tile([CIN, CEXP], BF16, tag="web")
        nc.vector.tensor_copy(web[:], we[:])

        wd = work.tile([CTX, NBLK * 9], F32, tag="wd")
        for blk in range(NBLK):
            mix(wd[:, blk * 9:(blk + 1) * 9],
                lambda k: w_depw[:, (blk * K + k) * 9:(blk * K + k + 1) * 9], 1, CTX)

        wp = work.tile([CTX, NBLK * COUT], F32, tag="wp")
        for blk in range(NBLK):
            mix(wp[:, blk * COUT:(blk + 1) * COUT],
                lambda k: w_projw[:, (blk * K + k) * COUT:(blk * K + k + 1) * COUT],
                2, CTX)
        wpb = work.tile([CTX, NBLK * COUT], BF16, tag="wpb")
        nc.vector.tensor_copy(wpb[:], wp[:])

        zs = []
        for blk in range(NBLK):
            r_blk = work.tile([CTX, FT], BF16, tag="sa")
            for ch in range(NCH):
                ps_e = ps_mm.tile([CTX, CHW], F32, tag="pse", bufs=3)
                nc.tensor.matmul(ps_e[:], web[:, blk * CTX:(blk + 1) * CTX],
                                 x0b[:, ch * CHW:(ch + 1) * CHW],
                                 start=True, stop=True)
                nc.scalar.activation(r_blk[:, ch * CHW:(ch + 1) * CHW], ps_e[:],
                                     AF.Relu, bias=w_eb3[:, blk:blk + 1], scale=1.0)
            v_blk = work.tile([CTX, FT], BF16, tag="sb")
            nc.vector.tensor_scalar(v_blk[:], r_blk[:], 6.0, 1.0 / 6.0, OP.min, OP.mult)
            xe = work.tile([CTX, NPAD], BF16, tag="xe")
            # zero only the pad regions (lead row, tail row, t-pad columns)
            nc.vector.memset(xe[:, 0:XOFF + TP], 0.0)
            nc.vector.memset(xe[:, XOFF + (F + 1) * TP:NPAD], 0.0)
            xep = xe[:, XOFF:XOFF + FP * TP]
            nc.vector.memset(
                xep.rearrange("p (f t) -> p f t", t=TP)[:, 1:1 + F, 0:TOFF], 0.0)
            nc.vector.memset(
                xep.rearrange("p (f t) -> p f t", t=TP)[:, 1:1 + F, TOFF + T:TP], 0.0)
            xe3 = xep.rearrange("p (f t) -> p f t", t=TP)
            nc.vector.scalar_tensor_tensor(
                xe3[:, 1:1 + F, TOFF:TOFF + T],
                r_blk.rearrange("p (f t) -> p f t", t=T), -3.0,
                v_blk.rearrange("p (f t) -> p f t", t=T), OP.add, OP.mult)

            # diag lhsT tiles for all 9 PE taps
            PET = [(df, dt) for df in (-1, 0, 1) for dt in (-1, 0, 1)]
            dg = work.tile([CTX, 9 * CTX], BF16, tag="dg")
            for i, (df, dt) in enumerate(PET):
                ti = (df + 1) * 3 + (dt + 1)
                nc.vector.tensor_scalar(
                    dg[:, i * CTX:(i + 1) * CTX], w_idb[0:CTX, 0:CTX],
                    wd[:, blk * 9 + ti: blk * 9 + ti + 1], None, OP.mult)

            z = zpool.tile([CTX, FT], BF16, tag=f"z{blk}")
            GF = 4  # f-rows per psum group (1 bank)
            for g in range(F // GF):
                ps_d = ps_mm.tile([CTX, GF * T], F32, tag="psd", bufs=3)
                for i, (df, dt) in enumerate(PET):
                    rv = xe3[:, 1 + g * GF + df: 1 + g * GF + df + GF,
                             TOFF + dt: TOFF + dt + T]
                    nc.tensor.matmul(ps_d[:], dg[:, i * CTX:(i + 1) * CTX], rv,
                                     start=(i == 0), stop=(i == 8))
                y1 = work.tile([CTX, GF * T], BF16, tag="y1")
                y2 = work.tile([CTX, GF * T], BF16, tag="y2")
                nc.scalar.activation(y1[:], ps_d[:], AF.Identity,
                                     bias=cj[2][:, blk * S + s: blk * S + s + 1],
                                     scale=cj[0][:, blk * S + s: blk * S + s + 1])
                nc.scalar.activation(y2[:], ps_d[:], AF.Identity,
                                     bias=cj[3][:, blk * S + s: blk * S + s + 1],
                                     scale=cj[1][:, blk * S + s: blk * S + s + 1])
                zsl = z[:, g * GF * T:(g + 1) * GF * T]
                nc.vector.tensor_tensor(zsl, y1[:], y2[:], OP.max)
                z3 = zsl.rearrange("p (f t) -> p f t", t=T)
                gf_v = sigf[:, (blk * S + s) * F + g * GF:
                            (blk * S + s) * F + (g + 1) * GF] \
                    .unsqueeze(2).broadcast_to((CTX, GF, T))
                nc.vector.tensor_tensor(z3[:], z3[:], gf_v, OP.mult)
                gt_v = sigt[:, (blk * S + s) * T:(blk * S + s + 1) * T] \
                    .unsqueeze(1).broadcast_to((CTX, GF, T))
                nc.gpsimd.tensor_tensor(z3[:], z3[:], gt_v, OP.mult)
            zs.append(z)

        for ch in range(NCH):
            ps_p = ps_mm.tile([COUT, CHW], F32, tag="psp")
            for blk in range(NBLK):
                nc.tensor.matmul(ps_p[:], wpb[:, blk * COUT:(blk + 1) * COUT],
                                 zs[blk][:, ch * CHW:(ch + 1) * CHW],
                                 start=(blk == 0), stop=(blk == NBLK - 1))
            tpo = work.tile([COUT, CHW], F32, tag="tpo")
            nc.scalar.activation(tpo[:], ps_p[:], AF.Identity,
                                 bias=w_pbias[:COUT, 0:1], scale=1.0)
            xr = xpool.tile([CIN, CHW], F32, tag="x0s")
            nc.sync.dma_start(xr[:], xs[s][:, ch * CHW:(ch + 1) * CHW])
            outs = work.tile([COUT, CHW], F32, tag="outs")
            nc.gpsimd.tensor_add(outs[:], tpo[:], xr[:])
            nc.sync.dma_start(
                yout[s][:, ch * CHW:(ch + 1) * CHW], outs[:])


def _host_prep(inputs):
    """Precompute packed/folded weight arrays (numpy, O(weights))."""
    p = {k: np.asarray(v, dtype=np.float32) for k, v in inputs.items()}
    inv_j = p["cg_joint_gamma"] / np.sqrt(p["cg_joint_var"] + EPS)
    sh_j = p["cg_joint_beta"] - p["cg_joint_mean"] * inv_j
    jlf = (p["cg_joint_w"].T * inv_j[None, :]) / T
    jlt = (p["cg_joint_w"].T * inv_j[None, :]) / F
    jb3 = (sh_j + 3.0)[:, None]

    cvf = np.ascontiguousarray(p["cg_convf_w"].T)
    cvt = np.ascontiguousarray(p["cg_convt_w"].T)
    bfb = np.ascontiguousarray(p["cg_convf_b"].reshape(NBLK, CTX).T)
    btb = np.ascontiguousarray(p["cg_convt_b"].reshape(NBLK, CTX).T)

    sc = 1.0 / ((F + T) * TEMP)
    arw0 = np.concatenate([p["exp_res_w"], p["dep_res_w"], p["proj_res_w"]], 0).T * sc
    arb0 = np.concatenate([p["exp_res_b"], p["dep_res_b"], p["proj_res_b"]]) / TEMP
    arw = np.ascontiguousarray(np.vstack([arw0, arb0[None, :]]))

    drw_r = p["dr_w"].reshape(CEXP, 2 * M, CTX).transpose(1, 0, 2)
    drw0 = drw_r.reshape(2 * M * CEXP, CTX).T / (F + T)
    drb_r = p["dr_b"].reshape(CEXP, 2 * M).T.reshape(-1)
    drw = np.ascontiguousarray(np.vstack([drw0, drb_r[None, :]]))

    inv_e = p["exp_bn_gamma"] / np.sqrt(p["exp_bn_var"] + EPS)
    sh_e = p["exp_bn_beta"] - p["exp_bn_mean"] * inv_e
    ew = (p["exp_weight"] * inv_e[None, :, None]).transpose(0, 2, 1)  # [K,80,480]
    expw = np.ascontiguousarray(ew.transpose(1, 0, 2).reshape(CIN, K * CEXP))
    eb3 = np.ascontiguousarray((sh_e + 3.0).reshape(NBLK, CTX).T)

    inv_d = p["dep_bn_gamma"] / np.sqrt(p["dep_bn_var"] + EPS)
    sh_d = p["dep_bn_beta"] - p["dep_bn_mean"] * inv_d
    dw = (p["dep_weight"] * inv_d[None, :, None, None]).reshape(K, CEXP, 9)
    dw_b = dw.reshape(K, NBLK, CTX, 9).transpose(2, 1, 0, 3)
    depw = np.ascontiguousarray(dw_b.reshape(CTX, NBLK * K * 9))
    bnbd = np.ascontiguousarray(sh_d.reshape(NBLK, CTX).T)

    inv_p = p["proj_bn_gamma"] / np.sqrt(p["proj_bn_var"] + EPS)
    sh_p = p["proj_bn_beta"] - p["proj_bn_mean"] * inv_p
    pw = p["proj_weight"] * inv_p[None, :, None]        # [K, 80, 480]
    pw_b = pw.transpose(2, 0, 1).reshape(NBLK, CTX, K, COUT).transpose(1, 0, 2, 3)
    projw = np.ascontiguousarray(pw_b.reshape(CTX, NBLK * K * COUT))
    pbias = sh_p[:, None]

    identf = np.eye(128, dtype=np.float32)
    return dict(jlf=jlf, jlt=jlt, jb3=jb3, cvf=cvf, cvt=cvt, bfb=bfb, btb=btb,
                arw=arw, drw=drw, expw=expw, eb3=eb3,
                depw=depw, bnbd=bnbd, projw=projw, pbias=pbias, identf=identf,
                vtag=np.zeros((1, _VTAG), np.float32),
                identb=np.eye(128).astype(ml_dtypes.bfloat16),
                ones1=np.ones((1, S), np.float32))


_BUILT = {}


def _build():
    if "nc" in _BUILT:
        return _BUILT["nc"]
    nc = bacc.Bacc("TRN2", target_bir_lowering=False, debug=False,
                   num_devices=NCORES)
    d = lambda n, s: nc.dram_tensor(n, list(s), F32, kind="ExternalInput").ap()
    io = [
        d("xs", (S, CIN, FT)),
        d("jlf", (CIN, CTX)), d("jlt", (CIN, CTX)), d("jb3", (CTX, 1)),
        d("cvf", (CTX, CEXP)), d("cvt", (CTX, CEXP)),
        d("bfb", (CTX, NBLK)), d("btb", (CTX, NBLK)),
        d("arw", (CTX + 1, 3 * K)),
        d("drw", (CTX + 1, 2 * M * CEXP)),
        d("expw", (CIN, K * CEXP)), d("eb3", (CTX, NBLK)),
        d("depw", (CTX, NBLK * K * 9)), d("bnbd", (CTX, NBLK)),
        d("projw", (CTX, NBLK * K * COUT)), d("pbias", (COUT, 1)),
        d("identf", (128, 128)), d("vtag", (1, _VTAG)),
        nc.dram_tensor("identb", [128, 128], BF16, kind="ExternalInput").ap(),
        d("ones1", (1, S)),
        nc.dram_tensor("y", [S, COUT, FT], F32, kind="ExternalOutput").ap(),
    ]
    from contextlib import ExitStack
    with tile.TileContext(nc) as tc:
        with ExitStack() as es:
            _emit(tc, io, es)
    nc.compile()
    _BUILT["nc"] = nc
    return nc


def _purge_stale_neff_cache():
    """The neuronx compile cache can key on the HLO signature alone; two
    different bass programs with identical IO signatures may collide.  The
    _VTAG input makes collisions with other versions of *this* kernel
    unlikely, but purge defensively so a stale NEFF can never be loaded."""
    import shutil
    base = os.path.expanduser("~/.neuron-compile-cache")
    tag = os.path.join(base, f".dyblock_vtag_{_VTAG}")
    if os.path.exists(base) and not os.path.exists(tag):
        shutil.rmtree(base, ignore_errors=True)
        os.makedirs(base, exist_ok=True)
        open(tag, "w").close()


def kernel(**inputs):
    _purge_stale_neff_cache()
    nc = _build()
    host = _host_prep(inputs)
    x = np.asarray(inputs["x"], dtype=np.float32).reshape(B, CIN, FT)
    in_maps = []
    for c in range(NCORES):
        m = {"xs": np.ascontiguousarray(x[c * S:(c + 1) * S])}
        m.update(host)
        in_maps.append(m)
    res = run_bass_kernel_spmd(nc, in_maps, list(range(NCORES)))
    out = np.concatenate([res.results[c]["y"] for c in range(NCORES)], axis=0)
    return out.reshape(B, COUT, F, T)


if __name__ == "__main__":
    import reference as ref
    inp = {k: np.asarray(v) for k, v in ref.setup_inputs().items()}
    got = kernel(**inp)
    from np_ref import forward_np
    exp = forward_np(inp)
    rel = np.abs(got - exp).max() / np.abs(exp).max()
    print("rel err vs np_ref:", rel)
